# revision 1
# baseline (speedup 1.0000x reference)
"""Causal multi-head attention (B=4, L=2048, D=1024, H=16, HD=64) on 8 TRN2
NeuronCores.

Sharding: core c handles batch b = c//2 and head-group g = c%2 (8 heads =
512 output dims). Attention is fully independent per (b, h); no collectives.

Per-core device kernel (all matmuls in float32r: full-rate fp32 storage,
~1e-4 relative precision):
  - Q^T, K^T computed with head_dim on partitions: QT[dim, l] = Wq_g @ X_b^T,
    via lhsT=Wq_g^T tiles, rhs=X_b^T tiles (both passed pre-transposed from
    host; host transposes are layout prep of the sharded inputs).
  - V in natural [l, dim] layout, augmented with a ones column per head so
    the PV matmul also accumulates the softmax denominators.
  - S^T[m, q] = K^T.T @ Q^T per (head, q-chunk 512, m-tile 128); blocks
    entirely above the causal diagonal are skipped; exp(0.25*s) runs on
    ScalarE with the scale fused; diagonal blocks are masked by a 0/1
    multiply after exp (exact zeros). No max-subtraction is needed: logits
    are O(10) so fp32 exp cannot overflow, matching the reference softmax
    up to rounding.
  - O^T_aug[65, q] accumulates over m-tiles in PSUM (ones column = row sums),
    then a PE transpose yields O[q, 65]; reciprocal of column 64 normalizes.
"""

import sys

if "/opt/trn_rl_repo" not in sys.path:
    sys.path.insert(0, "/opt/trn_rl_repo")

import numpy as np

import concourse.bass as bass  # noqa: F401  (engine types referenced via nc)
import concourse.bacc as bacc
import concourse.tile as tile
from concourse import mybir
from concourse.bass_utils import run_bass_kernel_spmd

B, L, D = 4, 2048, 1024
H, HD = 16, 64
NCORES = 8
DIMS = 512  # output dims per core (8 heads)
NKT = 8  # k-tiles over D
NDT = 4  # dim-tiles over DIMS
NQC = 4  # q-chunks of 512
NLT = 16  # l-tiles of 128
SCALE = 0.25  # 1/sqrt(H)
F32R = mybir.dt.float32r
F32 = mybir.dt.float32
AF = mybir.ActivationFunctionType

_cache = {}


def _build_kernel(sps_bufs=2, tp_in_sps=False, es_bufs=4, qk_bufs=2, outb_bufs=1, single_exp=False, v_bias=True):
    nc = bacc.Bacc("TRN2", target_bir_lowering=False, debug=False)

    XT = nc.declare_dram_parameter("XT", [D, L], F32R, isOutput=False)
    WQT = nc.declare_dram_parameter("WQT", [D, DIMS], F32R, isOutput=False)
    WKT = nc.declare_dram_parameter("WKT", [D, DIMS], F32R, isOutput=False)
    WVT = nc.declare_dram_parameter("WVT", [D, DIMS], F32R, isOutput=False)
    BQ = nc.declare_dram_parameter("BQ", [NDT, 128, 1], F32, isOutput=False)
    BK = nc.declare_dram_parameter("BK", [NDT, 128, 1], F32, isOutput=False)
    BV = nc.declare_dram_parameter("BV", [1, DIMS], F32R, isOutput=False)
    MASKS = nc.declare_dram_parameter("MASKS", [128, 128], F32R, isOutput=False)
    IDENT = nc.declare_dram_parameter("IDENT", [128, 128], F32, isOutput=False)
    ONES = nc.declare_dram_parameter("ONES", [128, 128], F32R, isOutput=False)
    OUT = nc.declare_dram_parameter("OUT", [L, DIMS], F32, isOutput=True)

    with tile.TileContext(nc) as tc:
        with tc.tile_pool(name="persist", bufs=1) as pp:
            # ---- resident tiles ----
            mask0 = pp.tile([128, 128], F32R, tag="mask0", name="mask0")
            nc.sync.dma_start(out=mask0, in_=MASKS[:, :])
            ident = pp.tile([128, 128], F32, tag="ident", name="ident")
            nc.sync.dma_start(out=ident, in_=IDENT[:, :])
            bq_sb = [pp.tile([128, 1], F32, tag=f"bq{d}", name=f"bq{d}") for d in range(NDT)]
            bk_sb = [pp.tile([128, 1], F32, tag=f"bk{d}", name=f"bk{d}") for d in range(NDT)]
            for d in range(NDT):
                nc.sync.dma_start(out=bq_sb[d], in_=BQ[d, :, :])
                nc.sync.dma_start(out=bk_sb[d], in_=BK[d, :, :])
            bv_sb = pp.tile([1, DIMS], F32R, tag="bv", name="bv")
            nc.sync.dma_start(out=bv_sb, in_=BV[:, :])
            ones = pp.tile([128, 128], F32R, tag="ones", name="ones")
            nc.sync.dma_start(out=ones, in_=ONES[:, :])

            # QT/KT: [128 (2 heads), 2048 l] per dim-tile; Vaug: [128 l, 8, 65]
            qt = [pp.tile([128, L], F32R, tag=f"qt{d}", name=f"qt{d}") for d in range(NDT)]
            # K^T per head, zero-padded to K=128 so every attention matmul
            # keeps the same PE row config (row-config switches cost ~400ns).
            ktp = [pp.tile([128, L], F32R, tag=f"ktp{h}", name=f"ktp{h}") for h in range(8)]
            vaug = [pp.tile([128, 8, 65], F32R, tag=f"va{t}", name=f"va{t}") for t in range(NLT)]

            # ---- phase B: QKV projections ----
            with (
                tc.tile_pool(name="wts", bufs=1) as wp,
                tc.tile_pool(name="xtp", bufs=8) as xtp,
                tc.tile_pool(name="psB", bufs=2, space="PSUM") as psB,
            ):
                wqt = [wp.tile([128, DIMS], F32R, tag=f"wq{k}", name=f"wq{k}") for k in range(NKT)]
                wkt = [wp.tile([128, DIMS], F32R, tag=f"wk{k}", name=f"wk{k}") for k in range(NKT)]
                wvt = [wp.tile([128, DIMS], F32R, tag=f"wv{k}", name=f"wv{k}") for k in range(NKT)]
                first_xts = []
                for k in range(NKT):
                    t = xtp.tile([128, 512], F32R, tag="xt", bufs=8, name="xt")
                    nc.sync.dma_start(out=t, in_=XT[k * 128 : (k + 1) * 128, 0:512])
                    first_xts.append(t)
                # weights stream on the scalar HWDGE queue in first-use
                # order (all wq, then wk, then wv) so Q matmuls start as soon
                # as the first xt tiles land; K/V weights arrive under the
                # shadow of Q/K compute.
                for k in range(NKT):
                    nc.scalar.dma_start(out=wqt[k], in_=WQT[k * 128 : (k + 1) * 128, :])
                for k in range(NKT):
                    nc.scalar.dma_start(out=wkt[k], in_=WKT[k * 128 : (k + 1) * 128, :])
                for k in range(NKT):
                    nc.scalar.dma_start(out=wvt[k], in_=WVT[k * 128 : (k + 1) * 128, :])

                for lc in range(NQC):  # 4 chunks of 512 l
                    lsl = slice(lc * 512, (lc + 1) * 512)
                    if lc == 0:
                        xts = first_xts
                    else:
                        xts = []
                        for k in range(NKT):
                            t = xtp.tile([128, 512], F32R, tag="xt", bufs=8, name="xt")
                            nc.sync.dma_start(
                                out=t, in_=XT[k * 128 : (k + 1) * 128, lsl]
                            )
                            xts.append(t)
                    for d in range(NDT):
                        dsl = slice(d * 128, (d + 1) * 128)
                        q_ps = psB.tile([128, 512], F32, tag="psq", bufs=qk_bufs, name="psq")
                        for k in range(NKT):
                            nc.tensor.matmul(
                                q_ps,
                                wqt[k][:, dsl],
                                xts[k][:],
                                start=(k == 0),
                                stop=(k == NKT - 1),
                            )
                        nc.vector.tensor_scalar_add(qt[d][:, lsl], q_ps, bq_sb[d][:])
                        k_ps = psB.tile([128, 512], F32, tag="psk", bufs=qk_bufs, name="psk")
                        for k in range(NKT):
                            nc.tensor.matmul(
                                k_ps,
                                wkt[k][:, dsl],
                                xts[k][:],
                                start=(k == 0),
                                stop=(k == NKT - 1),
                            )
                        nc.vector.tensor_scalar_add(
                            ktp[2 * d][0:64, lsl], k_ps[0:64, :], bk_sb[d][0:64]
                        )
                        nc.vector.tensor_scalar_add(
                            ktp[2 * d + 1][64:128, lsl], k_ps[64:128, :], bk_sb[d][64:128]
                        )
                        nc.vector.tensor_scalar_mul(
                            ktp[2 * d][64:128, lsl], k_ps[64:128, :], 0.0
                        )
                        nc.vector.tensor_scalar_mul(
                            ktp[2 * d + 1][0:64, lsl], k_ps[0:64, :], 0.0
                        )
                    # V natural: [l 128, dim 512] per l-subtile. The K=1 bias
                    # matmuls are batched up front so the PE row config only
                    # changes twice per chunk.
                    v_pss = []
                    for lb in range(4):
                        v_ps = psB.tile([128, 512], F32, tag="psv", bufs=4, name="psv")
                        if v_bias:
                            nc.tensor.matmul(
                                v_ps, ones[0:1, :], bv_sb[:, :], start=True, stop=False
                            )
                        v_pss.append(v_ps)
                    for lb in range(4):
                        lt = lc * 4 + lb
                        v_ps = v_pss[lb]
                        for k in range(NKT):
                            nc.tensor.matmul(
                                v_ps,
                                xts[k][:, lb * 128 : (lb + 1) * 128],
                                wvt[k][:],
                                start=(not v_bias and k == 0),
                                stop=(k == NKT - 1),
                            )
                        nc.vector.tensor_copy(
                            vaug[lt][:, :, 0:64],
                            v_ps[:].rearrange("p (h d) -> p h d", h=8),
                        )
                        nc.vector.tensor_copy(vaug[lt][:, :, 64:65], ones[:, 0:8].rearrange("p (h o) -> p h o", o=1))

            # ---- phase C: attention ----
            # Causal raggedness: block (qc, mt) only touches query columns
            # q' >= o where o = clamp(mt*128 - qc*512, 0, ..); within the
            # surviving range only the first 128 columns are triangular.
            with (
                tc.tile_pool(name="psS", bufs=sps_bufs, space="PSUM") as psS,
                tc.tile_pool(name="psO", bufs=1, space="PSUM") as psO,
                tc.tile_pool(name="psT", bufs=2, space="PSUM") as psT,
                tc.tile_pool(name="esb", bufs=es_bufs) as esb,
                tc.tile_pool(name="fin", bufs=2) as fin,
            ):
                for qc in range(NQC):
                    qsl0 = qc * 512
                    outb = [
                        fin.tile([128, DIMS], F32, tag=f"outb{qb}", bufs=outb_bufs, name=f"outb{qb}") for qb in range(4)
                    ]
                    nmt = 4 * qc + 4  # m-tiles below/at the diagonal
                    ots = []
                    for hp in range(4):  # head pairs (2hp, 2hp+1)
                        po_a = psO.tile([65, 512], F32, tag="poa", name="poa")
                        po_b = psO.tile([65, 512], F32, tag="pob", name="pob")
                        for mt in range(nmt):
                            msl = slice(mt * 128, (mt + 1) * 128)
                            off = mt * 128 - qc * 512
                            o = max(0, off)
                            vsa = slice(o, 512)
                            vsb = slice(512 + o, 1024)
                            qv = slice(qsl0 + o, qsl0 + 512)
                            s_ps = psS.tile([128, 1024], F32, tag="sps", name="sps")
                            nc.tensor.matmul(
                                s_ps[:, vsa],
                                ktp[2 * hp][:, msl],
                                qt[hp][:, qv],
                                start=True,
                                stop=True,
                            )
                            nc.tensor.matmul(
                                s_ps[:, vsb],
                                ktp[2 * hp + 1][:, msl],
                                qt[hp][:, qv],
                                start=True,
                                stop=True,
                            )
                            es = esb.tile([128, 1024], F32R, tag="es", name="es")
                            if o <= 128 or single_exp:
                                nc.scalar.activation(
                                    es[:, o:1024], s_ps[:, o:1024], AF.Exp, scale=SCALE
                                )
                            else:
                                nc.scalar.activation(
                                    es[:, vsa], s_ps[:, vsa], AF.Exp, scale=SCALE
                                )
                                nc.scalar.activation(
                                    es[:, vsb], s_ps[:, vsb], AF.Exp, scale=SCALE
                                )
                            if off >= 0:  # triangular 128-col edge of the block
                                w = min(o + 128, 512) - o
                                nc.vector.tensor_mul(
                                    es[:, o : o + w], es[:, o : o + w], mask0[:, 0:w]
                                )
                                nc.vector.tensor_mul(
                                    es[:, 512 + o : 512 + o + w],
                                    es[:, 512 + o : 512 + o + w],
                                    mask0[:, 0:w],
                                )
                            nc.tensor.matmul(
                                po_a[:, vsa],
                                vaug[mt][:, 2 * hp, :],
                                es[:, vsa],
                                start=(mt == 0),
                                stop=(mt == nmt - 1),
                            )
                            nc.tensor.matmul(
                                po_b[:, slice(o, 512)],
                                vaug[mt][:, 2 * hp + 1, :],
                                es[:, vsb],
                                start=(mt == 0),
                                stop=(mt == nmt - 1),
                            )
                        for half, po in ((0, po_a), (1, po_b)):
                            h = 2 * hp + half
                            ot = fin.tile(
                                [65, 512], F32, tag=f"ot{h}", bufs=1, name=f"ot{h}"
                            )
                            nc.vector.tensor_copy(ot, po)
                            ots.append((h, ot))
                    # batched finalize: all 32 transposes in one PE config run
                    for h, ot in ots:
                        for qb in range(4):
                            tp = (
                                psS.tile([128, 65], F32, tag="sps", name="tp")
                                if tp_in_sps
                                else psT.tile([128, 65], F32, tag="tp", name="tp")
                            )
                            nc.tensor.transpose(
                                tp,
                                ot[:, qb * 128 : (qb + 1) * 128],
                                ident[0:65, 0:65],
                            )
                            r = fin.tile([128, 1], F32, tag="r", name="r")
                            nc.vector.reciprocal(r, tp[:, 64:65])
                            nc.vector.tensor_scalar_mul(
                                outb[qb][:, h * 64 : (h + 1) * 64],
                                tp[:, 0:64],
                                r[:],
                            )
                    for qb in range(4):
                        row0 = qc * 512 + qb * 128
                        nc.sync.dma_start(
                            out=OUT[row0 : row0 + 128, :], in_=outb[qb][:]
                        )

    nc.compile()
    return nc


def _host_inputs(X, Wq, bq, Wk, bk, Wv, bv):
    """Build the 8 per-core input maps (host-side sharding + layout prep)."""
    X = np.asarray(X, dtype=np.float32)
    Wq = np.asarray(Wq, dtype=np.float32)
    Wk = np.asarray(Wk, dtype=np.float32)
    Wv = np.asarray(Wv, dtype=np.float32)
    bq = np.asarray(bq, dtype=np.float32)
    bk = np.asarray(bk, dtype=np.float32)
    bv = np.asarray(bv, dtype=np.float32)

    mask = (
        np.arange(128)[None, :] >= np.arange(128)[:, None]
    ).astype(np.float32)
    ident = np.eye(128, dtype=np.float32)

    in_maps = []
    for c in range(NCORES):
        b, g = divmod(c, 2)
        dsl = slice(g * DIMS, (g + 1) * DIMS)
        in_maps.append(
            {
                "XT": np.ascontiguousarray(X[b].T),
                "WQT": np.ascontiguousarray(Wq[dsl, :].T),
                "WKT": np.ascontiguousarray(Wk[dsl, :].T),
                "WVT": np.ascontiguousarray(Wv[dsl, :].T),
                "BQ": np.ascontiguousarray(bq[dsl].reshape(NDT, 128, 1)),
                "BK": np.ascontiguousarray(bk[dsl].reshape(NDT, 128, 1)),
                "BV": np.ascontiguousarray(bv[dsl].reshape(1, DIMS)),
                "MASKS": mask,
                "IDENT": ident,
                "ONES": np.ones((128, 128), dtype=np.float32),
            }
        )
    return in_maps


def _run(in_maps, trace=False, variant=None):
    key = ("nc", variant)
    if key not in _cache:
        kw = dict(VARIANTS.get(variant, {}))
        _cache[key] = _build_kernel(**kw)
    res = run_bass_kernel_spmd(
        _cache[key], in_maps, core_ids=list(range(NCORES)), trace=trace
    )
    return res


VARIANTS = {
    None: {},
    "sps3": {"sps_bufs": 3, "tp_in_sps": True},
    "esb5": {"es_bufs": 5},
    "esb3": {"es_bufs": 3},
    "psqk3": {"qk_bufs": 3},
    "outb2": {"outb_bufs": 2},
    "singleexp": {"single_exp": True},
    "nobias": {"v_bias": False},
}


def kernel(X, Wq, bq, Wk, bk, Wv, bv):
    in_maps = _host_inputs(X, Wq, bq, Wk, bk, Wv, bv)
    res = _run(in_maps, trace=False)
    out = np.empty((B, L, D), dtype=np.float32)
    for c in range(NCORES):
        b, g = divmod(c, 2)
        out[b, :, g * DIMS : (g + 1) * DIMS] = res.results[c]["OUT"]
    return out



# revision 12
# speedup vs baseline: 1.3427x; 1.3427x over previous
"""Causal multi-head attention (B=4, L=2048, D=1024, H=16, HD=64) on 8 TRN2
NeuronCores.

Sharding: core c handles batch b = c//2 and head-group g = c%2 (8 heads =
512 output dims). Attention is fully independent per (b, h); no collectives.

v2 design (vs. v1's fp32r + transposed-PV):
  - All matmul operands bf16 (1 cycle/row on the PE at ANY output width;
    fp32r drops to 4 cycles/row below 256-wide outputs). PSUM accumulation
    stays fp32. Output stays fp32.
  - PV computes O directly: out[q,65] = es[:,qtile].T @ Vaug[m,65] per
    (head, q-tile-128, m-tile-128), streaming only 65 V columns per matmul.
    Halves PV cycles vs streaming q, and removes every PE transpose
    (the ones-column of Vaug still accumulates softmax denominators).
  - The QKV projections of chunk lc+1 are interleaved into attention(qc=lc)'s
    PE instruction stream: the Activation engine (exp) is the co-bottleneck,
    and the interleave keeps the PE busy while Act drains. Attention(qc=0)
    starts as soon as the needed Q/K dim-tiles of chunk 0 are done.
  - Engine placement: exp on Act; Q/K/V PSUM->SBUF bias-copies on DVE;
    causal-mask multiplies and the final denominator-scaling on GpSimd
    (otherwise idle); reciprocals on DVE.
"""

import sys

if "/opt/trn_rl_repo" not in sys.path:
    sys.path.insert(0, "/opt/trn_rl_repo")

import ml_dtypes
import numpy as np

import concourse.bass as bass  # noqa: F401
import concourse.bacc as bacc
import concourse.tile as tile
from concourse import mybir
from concourse.bass_utils import run_bass_kernel_spmd

B, L, D = 4, 2048, 1024
H, HD = 16, 64
NCORES = 8
DIMS = 512  # output dims per core (8 heads)
NKT = 8  # k-tiles over D
NDT = 4  # dim-tiles over DIMS
NQC = 4  # q-chunks of 512
NLT = 16  # l-tiles of 128
SCALE = 0.25  # 1/sqrt(H)
BF16 = mybir.dt.bfloat16
F32 = mybir.dt.float32
AF = mybir.ActivationFunctionType

_cache = {}


def _build_kernel(sps_bufs=2, es_bufs=34, qkv_bufs=2, po_bufs=1,
                  masks_on="gpsimd", fin_on="vector"):
    nc = bacc.Bacc("TRN2", target_bir_lowering=False, debug=False)

    XT = nc.declare_dram_parameter("XT", [D, L], BF16, isOutput=False)
    WQT = nc.declare_dram_parameter("WQT", [D, DIMS], BF16, isOutput=False)
    WKT = nc.declare_dram_parameter("WKT", [D, DIMS], BF16, isOutput=False)
    WVT = nc.declare_dram_parameter("WVT", [D, DIMS], BF16, isOutput=False)
    BQ = nc.declare_dram_parameter("BQ", [NDT, 128, 1], F32, isOutput=False)
    BK = nc.declare_dram_parameter("BK", [NDT, 128, 1], F32, isOutput=False)
    BV = nc.declare_dram_parameter("BV", [1, DIMS], BF16, isOutput=False)
    MASKS = nc.declare_dram_parameter("MASKS", [128, 128], BF16, isOutput=False)
    OUT = nc.declare_dram_parameter("OUT", [L, DIMS], F32, isOutput=True)

    mask_eng = {"gpsimd": "gpsimd", "vector": "vector"}[masks_on]
    fin_eng = {"gpsimd": "gpsimd", "vector": "vector"}[fin_on]

    with tile.TileContext(nc) as tc:
        with tc.tile_pool(name="persist", bufs=1) as pp:
            # ---- resident tiles ----
            mask0 = pp.tile([128, 128], BF16, tag="mask0", name="mask0")
            nc.sync.dma_start(out=mask0, in_=MASKS[:, :])
            bq_sb = [pp.tile([128, 1], F32, tag=f"bq{d}", name=f"bq{d}") for d in range(NDT)]
            bk_sb = [pp.tile([128, 1], F32, tag=f"bk{d}", name=f"bk{d}") for d in range(NDT)]
            for d in range(NDT):
                nc.sync.dma_start(out=bq_sb[d], in_=BQ[d, :, :])
                nc.sync.dma_start(out=bk_sb[d], in_=BK[d, :, :])
            bv_sb = pp.tile([1, DIMS], BF16, tag="bv", name="bv")
            nc.sync.dma_start(out=bv_sb, in_=BV[:, :])
            ones1 = pp.tile([1, 128], BF16, tag="ones1", name="ones1")
            nc.gpsimd.memset(ones1[0:1, :], 1.0)
            vb_sb = pp.tile([128, DIMS], BF16, tag="vbb", name="vbb")

            # QT/KT: [128 (2 heads), 2048 l] per dim-tile; Vaug: [128 l, 8, 65]
            qt = [pp.tile([128, L], BF16, tag=f"qt{d}", name=f"qt{d}") for d in range(NDT)]
            # K^T per head, zero-padded to K=128 so every attention matmul
            # keeps the same PE row config. Zero-fills are split DVE/GpSimd
            # and ordered by first-use deadline (S of hp=h needs ktp[2h,2h+1]).
            ktp = [pp.tile([128, L], BF16, tag=f"ktp{h}", name=f"ktp{h}") for h in range(8)]
            vaug = [pp.tile([128, 8, 65], BF16, tag=f"va{t}", name=f"va{t}") for t in range(NLT)]

            def _ktp_pad(h):
                return ktp[h][64:128, :] if h % 2 == 0 else ktp[h][0:64, :]

            for h in (0, 1, 4, 5):
                nc.vector.memset(_ktp_pad(h), 0.0)

            with (
                tc.tile_pool(name="wts", bufs=1) as wp,
                tc.tile_pool(name="xtp", bufs=16) as xtp,
                tc.tile_pool(name="psQ", bufs=qkv_bufs, space="PSUM") as psQ,
                tc.tile_pool(name="psS", bufs=sps_bufs, space="PSUM") as psS,
                tc.tile_pool(name="psPO", bufs=po_bufs, space="PSUM") as psPO,
                tc.tile_pool(name="esb", bufs=es_bufs) as esb,
                tc.tile_pool(name="fin", bufs=2) as fin,
            ):
                wqt = [wp.tile([128, DIMS], BF16, tag=f"wq{k}", name=f"wq{k}") for k in range(NKT)]
                wkt = [wp.tile([128, DIMS], BF16, tag=f"wk{k}", name=f"wk{k}") for k in range(NKT)]
                wvt = [wp.tile([128, DIMS], BF16, tag=f"wv{k}", name=f"wv{k}") for k in range(NKT)]
                # weights stream on the scalar HWDGE queue (Act is idle
                # during the prologue) in first-use order.
                for k in range(NKT):
                    nc.scalar.dma_start(out=wqt[k], in_=WQT[k * 128 : (k + 1) * 128, :])
                for k in range(NKT):
                    nc.scalar.dma_start(out=wkt[k], in_=WKT[k * 128 : (k + 1) * 128, :])
                for k in range(NKT):
                    nc.scalar.dma_start(out=wvt[k], in_=WVT[k * 128 : (k + 1) * 128, :])

                xts = {}

                def load_chunk(lc):
                    lsl = slice(lc * 512, (lc + 1) * 512)
                    ts = []
                    for k in range(NKT):
                        t = xtp.tile([128, 512], BF16, tag="xt", bufs=16, name="xt")
                        nc.sync.dma_start(out=t, in_=XT[k * 128 : (k + 1) * 128, lsl])
                        ts.append(t)
                    xts[lc] = ts

                load_chunk(0)
                load_chunk(1)

                # V bias broadcast to all 128 partitions: vb = ones^T @ bv.
                vb_ps = psQ.tile([128, DIMS], F32, tag="qkv", bufs=qkv_bufs, name="vbps")
                nc.tensor.matmul(vb_ps, ones1[0:1, :], bv_sb[0:1, :], start=True, stop=True)
                nc.vector.tensor_copy(vb_sb, vb_ps)
                for h in (2, 3, 6, 7):
                    nc.gpsimd.memset(_ktp_pad(h), 0.0)
                for t in range(NLT):
                    nc.gpsimd.memset(vaug[t][:, :, 64:65], 1.0)

                # ---- QKV projection units (each: 8 PE matmuls + DVE copy) ----
                def emit_q(lc, d):
                    lsl = slice(lc * 512, (lc + 1) * 512)
                    dsl = slice(d * 128, (d + 1) * 128)
                    ps = psQ.tile([128, 512], F32, tag="qkv", bufs=qkv_bufs, name="psq")
                    for k in range(NKT):
                        nc.tensor.matmul(ps, wqt[k][:, dsl], xts[lc][k][:],
                                         start=(k == 0), stop=(k == NKT - 1))
                    nc.vector.tensor_scalar_add(qt[d][:, lsl], ps, bq_sb[d][:])

                def emit_k(lc, d):
                    lsl = slice(lc * 512, (lc + 1) * 512)
                    dsl = slice(d * 128, (d + 1) * 128)
                    ps = psQ.tile([128, 512], F32, tag="qkv", bufs=qkv_bufs, name="psk")
                    for k in range(NKT):
                        nc.tensor.matmul(ps, wkt[k][:, dsl], xts[lc][k][:],
                                         start=(k == 0), stop=(k == NKT - 1))
                    nc.vector.tensor_scalar_add(ktp[2 * d][0:64, lsl], ps[0:64, :], bk_sb[d][0:64])
                    nc.vector.tensor_scalar_add(ktp[2 * d + 1][64:128, lsl], ps[64:128, :], bk_sb[d][64:128])

                def emit_v(lc, lb):
                    lt = lc * 4 + lb
                    ps = psQ.tile([128, 512], F32, tag="qkv", bufs=qkv_bufs, name="psv")
                    for k in range(NKT):
                        nc.tensor.matmul(ps, xts[lc][k][:, lb * 128 : (lb + 1) * 128], wvt[k][:],
                                         start=(k == 0), stop=(k == NKT - 1))
                    nc.vector.tensor_add(
                        vaug[lt][:, :, 0:64],
                        ps[:].rearrange("p (h d) -> p h d", h=8),
                        vb_sb[:].rearrange("p (h d) -> p h d", h=8),
                    )

                # ---- attention scheduler ----
                # PSUM constraint: within one bank only ONE matmul accumulation
                # group may be open at a time (an open group's partial is
                # dropped when another region of the same bank starts).  So PV
                # groups are emitted as CONTIGUOUS per-bank runs, deferred by
                # one head-pair: while S/exp of (qc,hp) stream, the PV groups
                # of the previous head-pair (whose es tiles persist) are
                # emitted between the S matmuls, one open group per po bank.
                pv_queue = []  # deferred closures (PV groups / finalizes)
                feng = getattr(nc, fin_eng)
                meng = getattr(nc, mask_eng)

                def push_hp_pv(qc, hp, es_list, outb):
                    po_a = psPO.tile([128, 4, 65], F32, tag="poa", bufs=po_bufs, name="poa")
                    po_b = psPO.tile([128, 4, 65], F32, tag="pob", bufs=po_bufs, name="pob")

                    def group(qb, half, po):
                        def emit():
                            last = 4 * qc + qb
                            for mt in range(last + 1):
                                nc.tensor.matmul(
                                    po[:, qb, :],
                                    es_list[mt][:, 512 * half + qb * 128 : 512 * half + (qb + 1) * 128],
                                    vaug[mt][:, 2 * hp + half, :],
                                    start=(mt == 0), stop=(mt == last))
                        return emit

                    for qb in range(4):
                        pv_queue.append(group(qb, 0, po_a))
                        pv_queue.append(group(qb, 1, po_b))

                    def finalize():
                        for half, po in ((0, po_a), (1, po_b)):
                            h = 2 * hp + half
                            r = fin.tile([128, 4, 1], F32, tag="r", bufs=4, name="r")
                            nc.vector.reciprocal(r, po[:, :, 64:65])
                            for qb in range(4):
                                feng.tensor_scalar_mul(
                                    outb[qb][:, h * 64 : (h + 1) * 64],
                                    po[:, qb, 0:64], r[:, qb, :])
                        if hp == 3:
                            for qb in range(4):
                                row0 = qc * 512 + qb * 128
                                nc.sync.dma_start(out=OUT[row0 : row0 + 128, :], in_=outb[qb][:])
                    pv_queue.append(finalize)

                def emit_segment(qc, pre_units, slot_units, spread_units):
                    """pre_units: {hp: [unit,...]} emitted at that hp's start.
                    slot_units: {global_slot_idx: [unit,...]}.
                    spread_units: list spread evenly over all slots."""
                    nmt = 4 * qc + 4
                    total_slots = 4 * nmt
                    n_spread = len(spread_units)
                    spread_at = set()
                    if n_spread:
                        for i in range(n_spread):
                            spread_at.add(int((i + 0.5) * total_slots / n_spread))
                    spread_iter = iter(spread_units)
                    outb = [fin.tile([128, DIMS], F32, tag=f"outb{qb}", bufs=2,
                                     name=f"outb{qb}") for qb in range(4)]
                    slot = 0
                    for hp in range(4):
                        for u in pre_units.get(hp, ()):
                            u()
                        # drain rate: finish the deferred queue within this block
                        pops = (len(pv_queue) + nmt - 1) // nmt
                        es_list = []
                        for mt in range(nmt):
                            msl = slice(mt * 128, (mt + 1) * 128)
                            off = mt * 128 - qc * 512
                            o = max(0, off)
                            qa = slice(qc * 512 + o, (qc + 1) * 512)
                            s_ps = psS.tile([128, 1024], F32, tag="sps", bufs=sps_bufs, name="sps")
                            nc.tensor.matmul(s_ps[:, o:512], ktp[2 * hp][:, msl],
                                             qt[hp][:, qa], start=True, stop=True)
                            nc.tensor.matmul(s_ps[:, 512 + o : 1024], ktp[2 * hp + 1][:, msl],
                                             qt[hp][:, qa], start=True, stop=True)
                            es = esb.tile([128, 1024], BF16, tag="es", bufs=es_bufs, name="es")
                            if o <= 128:
                                nc.scalar.activation(es[:, o:1024], s_ps[:, o:1024], AF.Exp, scale=SCALE)
                            else:
                                nc.scalar.activation(es[:, o:512], s_ps[:, o:512], AF.Exp, scale=SCALE)
                                nc.scalar.activation(es[:, 512 + o : 1024], s_ps[:, 512 + o : 1024],
                                                     AF.Exp, scale=SCALE)
                            if off >= 0:  # triangular 128-col edge of the block
                                meng.tensor_mul(es[:, o : o + 128], es[:, o : o + 128], mask0[:, :])
                                meng.tensor_mul(es[:, 512 + o : 512 + o + 128],
                                                es[:, 512 + o : 512 + o + 128], mask0[:, :])
                            es_list.append(es)
                            for u in slot_units.get(slot, ()):
                                u()
                            if slot in spread_at:
                                u = next(spread_iter, None)
                                if u is not None:
                                    u()
                            slot += 1
                            for _ in range(pops):
                                if pv_queue:
                                    pv_queue.pop(0)()
                        push_hp_pv(qc, hp, es_list, outb)

                # ---- schedule ----
                # seg qc=0: chunk-0 Q/K per hp as pre-units, V at hp0's first
                #           slots, chunk-1 units spread over the rest.
                emit_segment(
                    0,
                    pre_units={hp: [lambda d=hp: emit_q(0, d), lambda d=hp: emit_k(0, d)]
                               for hp in range(4)},
                    slot_units={lb: [lambda b=lb: emit_v(0, b)] for lb in range(4)},
                    spread_units=[lambda d=d: emit_q(1, d) for d in range(NDT)]
                    + [lambda d=d: emit_k(1, d) for d in range(NDT)]
                    + [lambda b=b: emit_v(1, b) for b in range(4)],
                )
                load_chunk(2)
                emit_segment(
                    1, pre_units={}, slot_units={},
                    spread_units=[lambda d=d: emit_q(2, d) for d in range(NDT)]
                    + [lambda d=d: emit_k(2, d) for d in range(NDT)]
                    + [lambda b=b: emit_v(2, b) for b in range(4)],
                )
                load_chunk(3)
                emit_segment(
                    2, pre_units={}, slot_units={},
                    spread_units=[lambda d=d: emit_q(3, d) for d in range(NDT)],
                )
                # chunk-3 K/V land in hp0's first 8 slots (needed from mt=12).
                kv3 = [lambda d=d: emit_k(3, d) for d in range(NDT)] \
                    + [lambda b=b: emit_v(3, b) for b in range(4)]
                emit_segment(
                    3, pre_units={}, slot_units={i: [kv3[i]] for i in range(8)},
                    spread_units=[],
                )
                # drain the deferred PV work of the last head pair
                for u in pv_queue:
                    u()
                pv_queue.clear()

    nc.compile()
    return nc


def _host_inputs(X, Wq, bq, Wk, bk, Wv, bv):
    """Build the 8 per-core input maps (host-side sharding + layout prep)."""
    X = np.asarray(X, dtype=np.float32)
    Wq = np.asarray(Wq, dtype=np.float32)
    Wk = np.asarray(Wk, dtype=np.float32)
    Wv = np.asarray(Wv, dtype=np.float32)
    bq = np.asarray(bq, dtype=np.float32)
    bk = np.asarray(bk, dtype=np.float32)
    bv = np.asarray(bv, dtype=np.float32)

    bf = ml_dtypes.bfloat16
    mask = (np.arange(128)[None, :] >= np.arange(128)[:, None]).astype(bf)

    in_maps = []
    for c in range(NCORES):
        b, g = divmod(c, 2)
        dsl = slice(g * DIMS, (g + 1) * DIMS)
        in_maps.append(
            {
                "XT": np.ascontiguousarray(X[b].T).astype(bf),
                "WQT": np.ascontiguousarray(Wq[dsl, :].T).astype(bf),
                "WKT": np.ascontiguousarray(Wk[dsl, :].T).astype(bf),
                "WVT": np.ascontiguousarray(Wv[dsl, :].T).astype(bf),
                "BQ": np.ascontiguousarray(bq[dsl].reshape(NDT, 128, 1)),
                "BK": np.ascontiguousarray(bk[dsl].reshape(NDT, 128, 1)),
                "BV": np.ascontiguousarray(bv[dsl].reshape(1, DIMS)).astype(bf),
                "MASKS": mask,
            }
        )
    return in_maps


def _run(in_maps, trace=False, variant=None):
    key = ("nc", variant)
    if key not in _cache:
        kw = dict(VARIANTS.get(variant, {}))
        _cache[key] = _build_kernel(**kw)
    res = run_bass_kernel_spmd(
        _cache[key], in_maps, core_ids=list(range(NCORES)), trace=trace
    )
    return res


VARIANTS = {
    None: {},
    "sps3": {"sps_bufs": 3},
    "po2": {"po_bufs": 2},
    "maskdve": {"masks_on": "vector"},
}


def kernel(X, Wq, bq, Wk, bk, Wv, bv):
    in_maps = _host_inputs(X, Wq, bq, Wk, bk, Wv, bv)
    res = _run(in_maps, trace=False)
    out = np.empty((B, L, D), dtype=np.float32)
    for c in range(NCORES):
        b, g = divmod(c, 2)
        out[b, :, g * DIMS : (g + 1) * DIMS] = res.results[c]["OUT"]
    return out


# revision 15
# speedup vs baseline: 1.3464x; 1.0028x over previous
"""Causal multi-head attention (B=4, L=2048, D=1024, H=16, HD=64) on 8 TRN2
NeuronCores.

Sharding: core c handles batch b = c//2 and head-group g = c%2 (8 heads =
512 output dims). Attention is fully independent per (b, h); no collectives.

v2 design (vs. v1's fp32r + transposed-PV):
  - All matmul operands bf16 (1 cycle/row on the PE at ANY output width;
    fp32r drops to 4 cycles/row below 256-wide outputs). PSUM accumulation
    stays fp32. Output stays fp32.
  - PV computes O directly: out[q,65] = es[:,qtile].T @ Vaug[m,65] per
    (head, q-tile-128, m-tile-128), streaming only 65 V columns per matmul.
    Halves PV cycles vs streaming q, and removes every PE transpose
    (the ones-column of Vaug still accumulates softmax denominators).
  - The QKV projections of chunk lc+1 are interleaved into attention(qc=lc)'s
    PE instruction stream: the Activation engine (exp) is the co-bottleneck,
    and the interleave keeps the PE busy while Act drains. Attention(qc=0)
    starts as soon as the needed Q/K dim-tiles of chunk 0 are done.
  - Engine placement: exp on Act; Q/K/V PSUM->SBUF bias-copies on DVE;
    causal-mask multiplies and the final denominator-scaling on GpSimd
    (otherwise idle); reciprocals on DVE.
"""

import sys

if "/opt/trn_rl_repo" not in sys.path:
    sys.path.insert(0, "/opt/trn_rl_repo")

import ml_dtypes
import numpy as np

import concourse.bass as bass  # noqa: F401
import concourse.bacc as bacc
import concourse.tile as tile
from concourse import mybir
from concourse.bass_utils import run_bass_kernel_spmd

B, L, D = 4, 2048, 1024
H, HD = 16, 64
NCORES = 8
DIMS = 512  # output dims per core (8 heads)
NKT = 8  # k-tiles over D
NDT = 4  # dim-tiles over DIMS
NQC = 4  # q-chunks of 512
NLT = 16  # l-tiles of 128
SCALE = 0.25  # 1/sqrt(H)
BF16 = mybir.dt.bfloat16
F32 = mybir.dt.float32
AF = mybir.ActivationFunctionType

_cache = {}


def _build_kernel(sps_bufs=2, es_bufs=34, qkv_bufs=2, po_bufs=1,
                  masks_on="gpsimd", fin_on="vector"):
    nc = bacc.Bacc("TRN2", target_bir_lowering=False, debug=False)

    XT = nc.declare_dram_parameter("XT", [D, L], BF16, isOutput=False)
    WQT = nc.declare_dram_parameter("WQT", [D, DIMS], BF16, isOutput=False)
    WKT = nc.declare_dram_parameter("WKT", [D, DIMS], BF16, isOutput=False)
    WVT = nc.declare_dram_parameter("WVT", [D, DIMS], BF16, isOutput=False)
    BQ = nc.declare_dram_parameter("BQ", [NDT, 128, 1], F32, isOutput=False)
    BK = nc.declare_dram_parameter("BK", [NDT, 128, 1], F32, isOutput=False)
    BV = nc.declare_dram_parameter("BV", [1, DIMS], BF16, isOutput=False)
    MASKS = nc.declare_dram_parameter("MASKS", [128, 128], BF16, isOutput=False)
    OUT = nc.declare_dram_parameter("OUT", [L, DIMS], F32, isOutput=True)

    mask_eng = {"gpsimd": "gpsimd", "vector": "vector"}[masks_on]
    fin_eng = {"gpsimd": "gpsimd", "vector": "vector"}[fin_on]

    with tile.TileContext(nc) as tc:
        with tc.tile_pool(name="persist", bufs=1) as pp:
            # ---- resident tiles ----
            mask0 = pp.tile([128, 128], BF16, tag="mask0", name="mask0")
            nc.sync.dma_start(out=mask0, in_=MASKS[:, :])
            bq_sb = [pp.tile([128, 1], F32, tag=f"bq{d}", name=f"bq{d}") for d in range(NDT)]
            bk_sb = [pp.tile([128, 1], F32, tag=f"bk{d}", name=f"bk{d}") for d in range(NDT)]
            for d in range(NDT):
                nc.sync.dma_start(out=bq_sb[d], in_=BQ[d, :, :])
                nc.sync.dma_start(out=bk_sb[d], in_=BK[d, :, :])
            bv_sb = pp.tile([1, DIMS], BF16, tag="bv", name="bv")
            nc.sync.dma_start(out=bv_sb, in_=BV[:, :])
            ones1 = pp.tile([1, 128], BF16, tag="ones1", name="ones1")
            nc.gpsimd.memset(ones1[0:1, :], 1.0)
            vb_sb = pp.tile([128, DIMS], BF16, tag="vbb", name="vbb")

            # QT/KT: [128 (2 heads), 2048 l] per dim-tile; Vaug: [128 l, 8, 65]
            qt = [pp.tile([128, L], BF16, tag=f"qt{d}", name=f"qt{d}") for d in range(NDT)]
            # K^T per head, zero-padded to K=128 so every attention matmul
            # keeps the same PE row config. Zero-fills are split DVE/GpSimd
            # and ordered by first-use deadline (S of hp=h needs ktp[2h,2h+1]).
            ktp = [pp.tile([128, L], BF16, tag=f"ktp{h}", name=f"ktp{h}") for h in range(8)]
            vaug = [pp.tile([128, 8, 65], BF16, tag=f"va{t}", name=f"va{t}") for t in range(NLT)]

            def _ktp_pad(h):
                return ktp[h][64:128, :] if h % 2 == 0 else ktp[h][0:64, :]

            for h in (0, 1, 4, 5):
                nc.vector.memset(_ktp_pad(h), 0.0)

            with (
                tc.tile_pool(name="wts", bufs=1) as wp,
                tc.tile_pool(name="xtp", bufs=16) as xtp,
                tc.tile_pool(name="psQ", bufs=qkv_bufs, space="PSUM") as psQ,
                tc.tile_pool(name="psS", bufs=sps_bufs, space="PSUM") as psS,
                tc.tile_pool(name="psPO", bufs=po_bufs, space="PSUM") as psPO,
                tc.tile_pool(name="esb", bufs=es_bufs) as esb,
                tc.tile_pool(name="fin", bufs=2) as fin,
            ):
                wqt = [wp.tile([128, DIMS], BF16, tag=f"wq{k}", name=f"wq{k}") for k in range(NKT)]
                wkt = [wp.tile([128, DIMS], BF16, tag=f"wk{k}", name=f"wk{k}") for k in range(NKT)]
                wvt = [wp.tile([128, DIMS], BF16, tag=f"wv{k}", name=f"wv{k}") for k in range(NKT)]
                # weights stream on the scalar HWDGE queue (Act is idle
                # during the prologue) in first-use order.
                for k in range(NKT):
                    nc.scalar.dma_start(out=wqt[k], in_=WQT[k * 128 : (k + 1) * 128, :])
                for k in range(NKT):
                    nc.scalar.dma_start(out=wkt[k], in_=WKT[k * 128 : (k + 1) * 128, :])
                for k in range(NKT):
                    nc.scalar.dma_start(out=wvt[k], in_=WVT[k * 128 : (k + 1) * 128, :])

                xts = {}

                def load_chunk(lc):
                    lsl = slice(lc * 512, (lc + 1) * 512)
                    ts = []
                    for k in range(NKT):
                        t = xtp.tile([128, 512], BF16, tag="xt", bufs=16, name="xt")
                        nc.sync.dma_start(out=t, in_=XT[k * 128 : (k + 1) * 128, lsl])
                        ts.append(t)
                    xts[lc] = ts

                load_chunk(0)
                load_chunk(1)

                # V bias broadcast to all 128 partitions: vb = ones^T @ bv.
                vb_ps = psQ.tile([128, DIMS], F32, tag="qkv", bufs=qkv_bufs, name="vbps")
                nc.tensor.matmul(vb_ps, ones1[0:1, :], bv_sb[0:1, :], start=True, stop=True)
                nc.vector.tensor_copy(vb_sb, vb_ps)
                for h in (2, 3, 6, 7):
                    nc.gpsimd.memset(_ktp_pad(h), 0.0)
                for t in range(NLT):
                    nc.gpsimd.memset(vaug[t][:, :, 64:65], 1.0)

                # ---- QKV projection units (each: 8 PE matmuls + DVE copy) ----
                def emit_q(lc, d):
                    lsl = slice(lc * 512, (lc + 1) * 512)
                    dsl = slice(d * 128, (d + 1) * 128)
                    ps = psQ.tile([128, 512], F32, tag="qkv", bufs=qkv_bufs, name="psq")
                    for k in range(NKT):
                        nc.tensor.matmul(ps, wqt[k][:, dsl], xts[lc][k][:],
                                         start=(k == 0), stop=(k == NKT - 1))
                    nc.vector.tensor_scalar_add(qt[d][:, lsl], ps, bq_sb[d][:])

                def emit_k(lc, d):
                    lsl = slice(lc * 512, (lc + 1) * 512)
                    dsl = slice(d * 128, (d + 1) * 128)
                    ps = psQ.tile([128, 512], F32, tag="qkv", bufs=qkv_bufs, name="psk")
                    for k in range(NKT):
                        nc.tensor.matmul(ps, wkt[k][:, dsl], xts[lc][k][:],
                                         start=(k == 0), stop=(k == NKT - 1))
                    nc.vector.tensor_scalar_add(ktp[2 * d][0:64, lsl], ps[0:64, :], bk_sb[d][0:64])
                    nc.vector.tensor_scalar_add(ktp[2 * d + 1][64:128, lsl], ps[64:128, :], bk_sb[d][64:128])

                def emit_v(lc, lb):
                    lt = lc * 4 + lb
                    ps = psQ.tile([128, 512], F32, tag="qkv", bufs=qkv_bufs, name="psv")
                    for k in range(NKT):
                        nc.tensor.matmul(ps, xts[lc][k][:, lb * 128 : (lb + 1) * 128], wvt[k][:],
                                         start=(k == 0), stop=(k == NKT - 1))
                    nc.vector.tensor_add(
                        vaug[lt][:, :, 0:64],
                        ps[:].rearrange("p (h d) -> p h d", h=8),
                        vb_sb[:].rearrange("p (h d) -> p h d", h=8),
                    )

                # ---- attention scheduler ----
                # PSUM constraint: within one bank only ONE matmul accumulation
                # group may be open at a time (an open group's partial is
                # dropped when another region of the same bank starts).  So PV
                # groups are emitted as CONTIGUOUS per-bank runs, deferred by
                # one head-pair: while S/exp of (qc,hp) stream, the PV groups
                # of the previous head-pair (whose es tiles persist) are
                # emitted between the S matmuls, one open group per po bank.
                pv_queue = []  # deferred closures (PV groups / finalizes)
                feng = getattr(nc, fin_eng)
                meng = getattr(nc, mask_eng)

                def push_hp_pv(qc, hp, es_list, outb):
                    # [128,4,128] = exactly one 2KB PSUM bank per tile: the
                    # one-open-accumulation-group-per-bank invariant must not
                    # depend on allocator packing.
                    po_a = psPO.tile([128, 4, 128], F32, tag="poa", bufs=po_bufs, name="poa")
                    po_b = psPO.tile([128, 4, 128], F32, tag="pob", bufs=po_bufs, name="pob")

                    def group(qb, half, po):
                        def emit():
                            last = 4 * qc + qb
                            for mt in range(last + 1):
                                nc.tensor.matmul(
                                    po[:, qb, 0:65],
                                    es_list[mt][:, 512 * half + qb * 128 : 512 * half + (qb + 1) * 128],
                                    vaug[mt][:, 2 * hp + half, :],
                                    start=(mt == 0), stop=(mt == last))
                        return emit

                    for qb in range(4):
                        pv_queue.append(group(qb, 0, po_a))
                        pv_queue.append(group(qb, 1, po_b))

                    def finalize():
                        for half, po in ((0, po_a), (1, po_b)):
                            h = 2 * hp + half
                            r = fin.tile([128, 4, 1], F32, tag="r", bufs=4, name="r")
                            nc.vector.reciprocal(r, po[:, :, 64:65])
                            for qb in range(4):
                                feng.tensor_scalar_mul(
                                    outb[qb][:, h * 64 : (h + 1) * 64],
                                    po[:, qb, 0:64], r[:, qb, :])
                        if hp == 3:
                            for qb in range(4):
                                row0 = qc * 512 + qb * 128
                                nc.sync.dma_start(out=OUT[row0 : row0 + 128, :], in_=outb[qb][:])
                    pv_queue.append(finalize)

                def emit_segment(qc, pre_units, slot_units, spread_units):
                    """pre_units: {hp: [unit,...]} emitted at that hp's start.
                    slot_units: {global_slot_idx: [unit,...]}.
                    spread_units: list spread evenly over all slots."""
                    nmt = 4 * qc + 4
                    total_slots = 4 * nmt
                    n_spread = len(spread_units)
                    spread_at = set()
                    if n_spread:
                        for i in range(n_spread):
                            spread_at.add(int((i + 0.5) * total_slots / n_spread))
                    spread_iter = iter(spread_units)
                    outb = [fin.tile([128, DIMS], F32, tag=f"outb{qb}", bufs=2,
                                     name=f"outb{qb}") for qb in range(4)]
                    slot = 0
                    for hp in range(4):
                        for u in pre_units.get(hp, ()):
                            u()
                        # drain rate: finish the deferred queue within this block
                        pops = (len(pv_queue) + nmt - 1) // nmt
                        es_list = []
                        for mt in range(nmt):
                            msl = slice(mt * 128, (mt + 1) * 128)
                            off = mt * 128 - qc * 512
                            o = max(0, off)
                            qa = slice(qc * 512 + o, (qc + 1) * 512)
                            s_ps = psS.tile([128, 1024], F32, tag="sps", bufs=sps_bufs, name="sps")
                            nc.tensor.matmul(s_ps[:, o:512], ktp[2 * hp][:, msl],
                                             qt[hp][:, qa], start=True, stop=True)
                            nc.tensor.matmul(s_ps[:, 512 + o : 1024], ktp[2 * hp + 1][:, msl],
                                             qt[hp][:, qa], start=True, stop=True)
                            es = esb.tile([128, 1024], BF16, tag="es", bufs=es_bufs, name="es")
                            if o <= 128:
                                nc.scalar.activation(es[:, o:1024], s_ps[:, o:1024], AF.Exp, scale=SCALE)
                            else:
                                nc.scalar.activation(es[:, o:512], s_ps[:, o:512], AF.Exp, scale=SCALE)
                                nc.scalar.activation(es[:, 512 + o : 1024], s_ps[:, 512 + o : 1024],
                                                     AF.Exp, scale=SCALE)
                            if off >= 0:  # triangular 128-col edge of the block
                                meng.tensor_mul(es[:, o : o + 128], es[:, o : o + 128], mask0[:, :])
                                meng.tensor_mul(es[:, 512 + o : 512 + o + 128],
                                                es[:, 512 + o : 512 + o + 128], mask0[:, :])
                            es_list.append(es)
                            for u in slot_units.get(slot, ()):
                                u()
                            if slot in spread_at:
                                u = next(spread_iter, None)
                                if u is not None:
                                    u()
                            slot += 1
                            for _ in range(pops):
                                if pv_queue:
                                    pv_queue.pop(0)()
                        push_hp_pv(qc, hp, es_list, outb)

                # ---- schedule ----
                # seg qc=0: chunk-0 Q/K for hp=0 as pre-units; hp 1..3's Q/K
                # prefetched one head-pair early via slots; V0 and chunk-1
                # units at explicit slots (deadline: before seg qc=1).
                q0 = [lambda d=d: emit_q(0, d) for d in range(NDT)]
                k0 = [lambda d=d: emit_k(0, d) for d in range(NDT)]
                v0 = [lambda b=b: emit_v(0, b) for b in range(4)]
                q1 = [lambda d=d: emit_q(1, d) for d in range(NDT)]
                k1 = [lambda d=d: emit_k(1, d) for d in range(NDT)]
                v1 = [lambda b=b: emit_v(1, b) for b in range(4)]
                emit_segment(
                    0,
                    pre_units={0: [q0[0], k0[0]]},
                    slot_units={
                        0: [v0[0]], 1: [q0[1]], 2: [k0[1], v0[1]], 3: [v0[2]],
                        4: [v0[3]], 5: [q0[2]], 6: [k0[2]], 7: [q1[0]],
                        8: [q1[1]], 9: [q0[3]], 10: [k0[3]], 11: [q1[2], q1[3]],
                        12: [k1[0], k1[1]], 13: [k1[2], k1[3]],
                        14: [v1[0], v1[1]], 15: [v1[2], v1[3]],
                    },
                    spread_units=[],
                )
                load_chunk(2)
                emit_segment(
                    1, pre_units={}, slot_units={},
                    spread_units=[lambda d=d: emit_q(2, d) for d in range(NDT)]
                    + [lambda d=d: emit_k(2, d) for d in range(NDT)]
                    + [lambda b=b: emit_v(2, b) for b in range(4)],
                )
                load_chunk(3)
                emit_segment(
                    2, pre_units={}, slot_units={},
                    spread_units=[lambda d=d: emit_q(3, d) for d in range(NDT)],
                )
                # chunk-3 K/V spread by first-use deadline: K3[d] feeds
                # S(hp=d, mt=12) at slot 16d+12; V3 feeds the PV groups of
                # hp=0 popped during hp=1's block (slot >= 16).
                k3 = [lambda d=d: emit_k(3, d) for d in range(NDT)]
                v3 = [lambda b=b: emit_v(3, b) for b in range(4)]
                emit_segment(
                    3, pre_units={},
                    slot_units={4: [k3[0]], 6: [v3[0]], 8: [v3[1]],
                                10: [v3[2]], 12: [v3[3]], 20: [k3[1]],
                                36: [k3[2]], 52: [k3[3]]},
                    spread_units=[],
                )
                # drain the deferred PV work of the last head pair
                for u in pv_queue:
                    u()
                pv_queue.clear()

    nc.compile()
    return nc


def _host_inputs(X, Wq, bq, Wk, bk, Wv, bv):
    """Build the 8 per-core input maps (host-side sharding + layout prep)."""
    X = np.asarray(X, dtype=np.float32)
    Wq = np.asarray(Wq, dtype=np.float32)
    Wk = np.asarray(Wk, dtype=np.float32)
    Wv = np.asarray(Wv, dtype=np.float32)
    bq = np.asarray(bq, dtype=np.float32)
    bk = np.asarray(bk, dtype=np.float32)
    bv = np.asarray(bv, dtype=np.float32)

    bf = ml_dtypes.bfloat16
    mask = (np.arange(128)[None, :] >= np.arange(128)[:, None]).astype(bf)

    in_maps = []
    for c in range(NCORES):
        b, g = divmod(c, 2)
        dsl = slice(g * DIMS, (g + 1) * DIMS)
        in_maps.append(
            {
                "XT": np.ascontiguousarray(X[b].T).astype(bf),
                "WQT": np.ascontiguousarray(Wq[dsl, :].T).astype(bf),
                "WKT": np.ascontiguousarray(Wk[dsl, :].T).astype(bf),
                "WVT": np.ascontiguousarray(Wv[dsl, :].T).astype(bf),
                "BQ": np.ascontiguousarray(bq[dsl].reshape(NDT, 128, 1)),
                "BK": np.ascontiguousarray(bk[dsl].reshape(NDT, 128, 1)),
                "BV": np.ascontiguousarray(bv[dsl].reshape(1, DIMS)).astype(bf),
                "MASKS": mask,
            }
        )
    return in_maps


def _run(in_maps, trace=False, variant=None):
    key = ("nc", variant)
    if key not in _cache:
        kw = dict(VARIANTS.get(variant, {}))
        _cache[key] = _build_kernel(**kw)
    res = run_bass_kernel_spmd(
        _cache[key], in_maps, core_ids=list(range(NCORES)), trace=trace
    )
    return res


VARIANTS = {
    None: {},
    "sps3": {"sps_bufs": 3},
    "po2": {"po_bufs": 2},
    "maskdve": {"masks_on": "vector"},
}


def kernel(X, Wq, bq, Wk, bk, Wv, bv):
    in_maps = _host_inputs(X, Wq, bq, Wk, bk, Wv, bv)
    res = _run(in_maps, trace=False)
    out = np.empty((B, L, D), dtype=np.float32)
    for c in range(NCORES):
        b, g = divmod(c, 2)
        out[b, :, g * DIMS : (g + 1) * DIMS] = res.results[c]["OUT"]
    return out
